# revision 72
# baseline (speedup 1.0000x reference)
"""Depth-guided 3x3 convolution (nn_DepthConv) on 8 TRN2 NeuronCores.

Sharding: data-parallel over batch (B=8 -> 1 image per core). Weights/bias
replicated. No collectives.

Per-core algorithm (image [C=64, H=128, W=128]):
  out[o,p] = bias[o] + sum_t W_t[o,c] * x[c, p+dt] * exp(-|d[p+dt]-d[p]|)

Layout (v2): columns packed at W=128 (NO column padding). Column wrap-around
of the dw=+-1 taps is killed by forcing the corresponding sim-map column to
zero (reference pads x with zeros, so a zero contribution is exact).
Rows split in halves on partitions: [0:64] = channels for image rows -1..64
(frame A), [64:128] = rows 63..128 (frame B); 66 frame rows -> FR = 66*128.
Frame row f=0 of A is the zero pad row; f=65 of B likewise.

With m_d[q] := exp(-|D(q+d)-D(q)|) (computed pixel-major [row, col] on 128
row-partitions, flattened into HBM scratch rows, then broadcast into the wp
tiles -- one DMA per map covers both halves via a 2-level partition AP):
  wp_d[q] = x[q+dl]*m_d[q]       (in-place over the broadcast map)
  wm_d[q] = x[q-dl]*m_d[q-dl]    (reads wp's pristine map slot shifted)
  center tap reads x directly (sim == 1).

Matmuls: per output chunk (4 rows, N=512) 9 fp16 matmuls (K=64) accumulate
into a [64, 512] PSUM tile; Act evacuates with fused bias into 4-chunk stage
tiles; SWDGE stores them. All bulk transfers use >=512B descriptors. The
pipeline runs in chunk-bands (small at both ends) so map-broadcast / x-load
/ multiply / matmul / store overlap and the PE stays saturated.
"""

import sys

sys.path.insert(0, "/opt/trn_rl_repo")

import numpy as np

import bass_rust
import concourse.bass as bass
import concourse.mybir as mybir
import concourse.bacc as bacc
import concourse.tile as tile
from concourse.bass_utils import run_bass_kernel_spmd
from concourse.masks import make_identity

F16 = mybir.dt.float16
F32 = mybir.dt.float32

C, O, H, W, KH, KW = 64, 64, 128, 128, 3, 3
ALPHA = 1.0
RH = 66               # frame rows per half (A: input rows -1..64, B: 63..128)
FR = RH * W           # 8448
DELTAS = [(0, 1), (1, -1), (1, 0), (1, 1)]
DLS = [dh * W + dw for dh, dw in DELTAS]   # 1, 127, 128, 129
ZCOL = {0: 127, 1: 0, 3: 127}              # forced zero column per map
NCH = 16              # output chunks per half (4 rows each)
Q0 = 128              # first output flat index (frame row 1)
QE = Q0 + NCH * 512   # 8320

# chunk-range bands (per half): small at both ends for pipeline ramp
BANDS = [(0, 3), (3, 5), (5, 8), (8, 10), (10, 12), (12, 14), (14, 16)]


def build_program():
    nc = bacc.Bacc("TRN2", target_bir_lowering=False, debug=False)

    x_t = nc.dram_tensor("x", [C, H, W], F32, kind="ExternalInput")
    d_t = nc.dram_tensor("depth", [1, H, W], F32, kind="ExternalInput")
    w_t = nc.dram_tensor("weight", [O, C, KH, KW], F32, kind="ExternalInput")
    b_t = nc.dram_tensor("bias", [O], F32, kind="ExternalInput")
    out_t = nc.dram_tensor("out", [O, H, W], F32, kind="ExternalOutput")
    # map scratch: row 2k = map k half A, row 2k+1 = half B (adjacent rows so
    # one flatten DMA writes both and one broadcast DMA reads both)
    scratch = nc.dram_tensor("mscratch", [8, FR], F16, kind="Internal")

    def bc_ap(k, w0, w1):
        """[128, w1-w0] read: scratch row 2k 64x broadcast, then row 2k+1."""
        base = scratch[2 * k, w0:w1]
        return bass_rust.AP(
            tensor=base.tensor, offset=base.offset,
            ap=[[FR, 2], [0, 64], [1, w1 - w0]],
        )

    with tile.TileContext(nc) as tc:
        with (
            tc.tile_pool(name="big", bufs=1) as big,
            tc.tile_pool(name="small", bufs=1) as small,
            tc.tile_pool(name="mapp", bufs=2) as mapp,
            tc.tile_pool(name="psum", bufs=6, space="PSUM") as psum_pool,
            tc.tile_pool(name="psumw", bufs=1, space="PSUM") as psumw_pool,
            tc.tile_pool(name="stage", bufs=2) as stage_pool,
        ):
            # ---------------- persistent SBUF tensors ----------------
            xbuf = big.tile([128, FR], F16, tag="xbuf")
            wplus = [
                big.tile([128, FR], F16, tag=f"wp{k}", name=f"wp{k}")
                for k in range(4)
            ]
            # wm_2 is merged with the center tap into K=128 matmul tiles:
            # TA = (wm_2^A | x_A), TB = (x_B | wm_2^B) -- the x copy sits on
            # the partition range where DVE/map-slot alignment works out.
            wminus = [
                big.tile([128, FR], F16, tag=f"wm{k}", name=f"wm{k}")
                if k != 2 else None
                for k in range(4)
            ]
            TA = big.tile([128, FR], F16, tag="TA")
            TB = big.tile([128, FR], F16, tag="TB")
            # slots 0-8: taps; slot 9: [W_-d2 ; W_cc] (TA), 10: [W_cc ; W_-d2]
            wT = small.tile([128, 11 * O], F16, tag="wT")
            # w_t loaded twice side by side: one [64,128] transpose per tap
            # (strided in-AP over both copies) fills both wT halves at once
            w_raw = small.tile([64, 2 * 576], F32, tag="wraw")
            bias_col = small.tile([64, 1], F32, tag="bias")
            dbuf = small.tile([128, W], F32, tag="dbuf")
            dsh = small.tile([128, W], F32, tag="dsh")
            ident = small.tile([128, 64], F32, tag="ident")
            zrow = small.tile([4, W], F16, tag="zrow")

            # ---------------- warm ACT tables (Abs/Exp) ----------------
            warm = small.tile([1, 8], F32, tag="warm")
            nc.vector.memset(warm[:, :], 0.0)
            nc.scalar.activation(
                warm[:, :], warm[:, :], mybir.ActivationFunctionType.Abs
            )
            nc.scalar.activation(
                warm[:, :], warm[:, :], mybir.ActivationFunctionType.Exp
            )

            # ---------------- maps path first (critical chain) ----------------
            # depth + weights on the sync queue (the warm-up blocks scalar
            # ~2.2us); everything here is small and must transfer early
            nc.sync.dma_start(out=dbuf[0:128, :], in_=d_t[0, :, :])
            nc.vector.memset(dsh[:, :], 0.0)
            nc.sync.dma_start(out=dsh[0:127, :], in_=d_t[0, 1:128, :])
            nc.sync.dma_start(out=w_raw[:, 0:576], in_=w_t[:, :, :, :])
            nc.sync.dma_start(out=w_raw[:, 576:1152], in_=w_t[:, :, :, :])
            nc.vector.memset(zrow[:, :], 0.0)
            nc.scalar.dma_start(
                out=bias_col[0:64, 0:1],
                in_=b_t[:].rearrange("(p o) -> p o", o=1),
            )

            # phase 1: all depth diffs back-to-back on DVE
            diffs = []
            for k, (dh, dw) in enumerate(DELTAS):
                dsrc = dsh if dh == 1 else dbuf
                a = max(0, -dw)
                b = min(W, W - dw)
                diff = mapp.tile([128, W], F32, tag=f"diff{k}", name=f"diff{k}")
                if dw != 0:
                    nc.vector.memset(diff[:, :], 0.0)
                nc.vector.tensor_sub(
                    diff[:, a:b], dsrc[:, a + dw : b + dw], dbuf[:, a:b]
                )
                diffs.append(diff)
            # x-load gate: zero the first element of EVERY frame row after
            # the subs on DVE. Every x load overlaps some row start (WAW), so
            # all of x queues at the DMA engines behind the latency-critical
            # map flatten/broadcast transfers. The cells are overwritten by
            # the loads (or by the halo zero rows).
            xrows = xbuf[:, :].rearrange("p (r w) -> p r w", r=RH)
            nc.vector.memset(xrows[:, :, 0:1], 0.0)
            tarows = TA[:, :].rearrange("p (r w) -> p r w", r=RH)
            nc.vector.memset(tarows[:, :, 0:1], 0.0)
            tbrows = TB[:, :].rearrange("p (r w) -> p r w", r=RH)
            nc.vector.memset(tbrows[:, :, 0:1], 0.0)

            # phase 2: abs+exp on Act, zero columns on DVE, flatten DMAs
            for k in range(4):
                absd = mapp.tile([128, W], F32, tag="absd")
                nc.scalar.activation(
                    absd[:, :], diffs[k][:, :], mybir.ActivationFunctionType.Abs
                )
                mt = mapp.tile([128, W], F16, tag=f"mt{k}")
                nc.scalar.activation(
                    mt[:, :], absd[:, :],
                    mybir.ActivationFunctionType.Exp, scale=-ALPHA,
                )
                if k in ZCOL:
                    nc.vector.memset(mt[:, ZCOL[k] : ZCOL[k] + 1], 0.0)

                # flatten to scratch rows 2k (half A, map rows -1..64 with
                # row -1 zeroed) and 2k+1 (half B, rows 63..128)
                dstA = scratch[2 * k : 2 * k + 1, 128:FR].rearrange(
                    "p (r w) -> p r w", r=65
                )
                dma = nc.sync if k % 2 == 0 else nc.scalar
                dma.dma_start(out=dstA[:, :, :], in_=mt[0:65, :])
                dstB = scratch[2 * k + 1 : 2 * k + 2, 0 : 65 * W].rearrange(
                    "p (r w) -> p r w", r=65
                )
                dma = nc.scalar if k % 2 == 0 else nc.sync
                dma.dma_start(out=dstB[:, :, :], in_=mt[63:128, :])

            # zero the 4 half-A slot heads (map "row -1") in one DMA
            zdst = scratch[0, 0:W]
            zout = bass_rust.AP(
                tensor=zdst.tensor, offset=zdst.offset, ap=[[2 * FR, 4], [1, W]]
            )
            nc.scalar.dma_start(out=zout, in_=zrow[0:4, :])

            # ---------------- weights -> lhsT [c, (t, o)] fp16 ----------------
            # duplicated load; one [64,128] transpose per tap fills both wT
            # partition halves (no SBUF->SBUF DMA, which would queue behind
            # bulk transfers on the DMA engines)
            make_identity(nc, ident[0:64, :])
            # all 9 transposes into 2 PSUM tiles (no pool-recycle stalls),
            # then bulk psum->wT copies on DVE (Act is busy with maps)
            wps1 = psumw_pool.tile([128, 5 * O], F32, tag="wtp1", name="wps1")
            wps2 = psumw_pool.tile([128, 4 * O], F32, tag="wtp2", name="wps2")
            for t in range(9):
                wr = w_raw[0:1, t : t + 1]
                win = bass_rust.AP(
                    tensor=wr.tensor, offset=wr.offset,
                    ap=[[1152, 64], [576, 2], [9, 64]],
                )
                dst = (
                    wps1[:, t * O : (t + 1) * O]
                    if t < 5
                    else wps2[:, (t - 5) * O : (t - 4) * O]
                )
                nc.tensor.transpose(dst, win, ident[0:64, :])
            nc.vector.tensor_copy(out=wT[:, 0 : 5 * O], in_=wps1[:, :])
            nc.vector.tensor_copy(out=wT[:, 5 * O : 9 * O], in_=wps2[:, :])
            # mixed slots for the merged (wm_2 | center) matmuls;
            # tap t=1 is W[-d2] (kh=0,kw=1), t=4 is the center
            nc.vector.tensor_copy(
                out=wT[0:64, 9 * O : 10 * O], in_=wps1[0:64, 1 * O : 2 * O]
            )
            nc.vector.tensor_copy(
                out=wT[64:128, 9 * O : 10 * O], in_=wps1[64:128, 4 * O : 5 * O]
            )
            nc.vector.tensor_copy(
                out=wT[0:64, 10 * O : 11 * O], in_=wps1[0:64, 4 * O : 5 * O]
            )
            nc.vector.tensor_copy(
                out=wT[64:128, 10 * O : 11 * O], in_=wps1[64:128, 1 * O : 2 * O]
            )

            # ---------------- halo zero rows / columns ----------------
            nc.gpsimd.memset(xbuf[0:64, 0:W], 0.0)            # A frame row 0
            nc.gpsimd.memset(xbuf[64:128, FR - W : FR], 0.0)  # B frame row 65
            # wp3 last output column: never written by its multiply but read
            # by matmuls; true value is 0 (map zero column kills the wrap)
            nc.gpsimd.memset(wplus[3][:, QE - 1 : QE], 0.0)
            # wm3[128] = x[-1]*m[-1] = 0: below the band-0 multiply range
            nc.gpsimd.memset(wminus[3][:, Q0 : DLS[3]], 0.0)


            # ---------------- banded pipeline ----------------
            xrow = [0]    # frame rows of x loaded so far (same for A and B)
            stages = [None, None]

            def load_x(upto_row):
                r0, r1 = xrow[0], min(RH, upto_row)
                if r1 <= r0:
                    return
                xrow[0] = r1
                ra0, ra1 = max(r0, 1), r1      # A rows with data (row 0 = pad)
                rb0, rb1 = r0, min(r1, 65)     # B rows with data (row 65 = pad)
                if ra0 == rb0 and ra1 == rb1:
                    # middle band: both halves in one DMA (2-level partitions)
                    xb = x_t[0, ra0 - 1, 0:W]
                    src = bass_rust.AP(
                        tensor=xb.tensor, offset=xb.offset,
                        ap=[[64 * W, 2], [H * W, 64], [W, ra1 - ra0], [1, W]],
                    )
                    nc.gpsimd.dma_start(
                        out=xbuf[0:128, ra0 * W : ra1 * W], in_=src
                    )
                else:
                    nc.gpsimd.dma_start(
                        out=xbuf[0:64, ra0 * W : ra1 * W],
                        in_=x_t[:, ra0 - 1 : ra1 - 1, :],
                    )
                    nc.gpsimd.dma_start(
                        out=xbuf[64:128, rb0 * W : rb1 * W],
                        in_=x_t[:, rb0 + 63 : rb1 + 63, :],
                    )
                # x copies for the merged-tap tiles (center-tap halves):
                # SBUF->SBUF from xbuf (no cast -> HWDGE queues, keeping the
                # Pool sequencer free for the xbuf loads)
                nc.sync.dma_start(
                    out=TA[64:128, ra0 * W : ra1 * W],
                    in_=xbuf[0:64, ra0 * W : ra1 * W],
                )
                nc.scalar.dma_start(
                    out=TB[0:64, rb0 * W : rb1 * W],
                    in_=xbuf[64:128, rb0 * W : rb1 * W],
                )

            bq = [0]  # broadcast cursor
            for bi, (j0, j1) in enumerate(BANDS):
                s = Q0 + 512 * j0
                e = Q0 + 512 * j1
                last = j1 == NCH

                # map broadcasts for window [bq, e): one DMA per map
                w0, w1 = bq[0], e
                bq[0] = e
                for k in range(4):
                    w1k = min(w1, QE - 1) if k == 3 else w1
                    dma = nc.sync if k % 2 == 0 else nc.scalar
                    dma.dma_start(
                        out=wplus[k][0:128, w0:w1k], in_=bc_ap(k, w0, w1k)
                    )

                # x rows needed: wp reads up to e + 129 -> row e//W + 2
                load_x(RH if last else e // W + 2)

                # wm_k[q] = x[q-dl]*m_k[q-dl], q in [s+dl, e+dl) (band-shifted
                # so reads hit the pristine map slot of this band's window)
                # band 0: per-chunk windows so the first matmuls start early
                subw = (
                    [(Q0 + 512 * j, Q0 + 512 * (j + 1)) for j in range(j0, j1)]
                    if bi == 0 else [(s, e)]
                )
                for swi, (s, e) in enumerate(subw):
                  for k in range(4):
                    dl = DLS[k]
                    a = max(Q0, dl) if (bi == 0 and swi == 0) else s + dl
                    b = min(e + dl, QE)
                    if k == 2:
                        # wm_2 halves live in TA[0:64] / TB[64:128]
                        nc.vector.tensor_tensor(
                            out=TA[0:64, a:b],
                            in0=xbuf[0:64, a - dl : b - dl],
                            in1=wplus[2][0:64, a - dl : b - dl],
                            op=mybir.AluOpType.mult,
                        )
                        nc.vector.tensor_tensor(
                            out=TB[64:128, a:b],
                            in0=xbuf[64:128, a - dl : b - dl],
                            in1=wplus[2][64:128, a - dl : b - dl],
                            op=mybir.AluOpType.mult,
                        )
                        continue
                    nc.vector.tensor_tensor(
                        out=wminus[k][:, a:b],
                        in0=xbuf[:, a - dl : b - dl],
                        in1=wplus[k][:, a - dl : b - dl],
                        op=mybir.AluOpType.mult,
                    )
                  # wp_k[q] = x[q+dl]*m_k[q] in place, q in [s, e)
                  for k in range(4):
                    dl = DLS[k]
                    b = min(e, FR - dl)
                    nc.vector.tensor_mul(
                        wplus[k][:, s:b], xbuf[:, s + dl : b + dl],
                        wplus[k][:, s:b],
                    )

                # matmuls + evac (+ store per 4-chunk group)
                for j in range(j0, j1):
                    q = Q0 + 512 * j
                    for h in range(2):
                        hr0 = 64 * h
                        # smaller groups toward the end -> short tail
                        gsz = 2 if j < 14 else 1
                        if j % gsz == 0:
                            stages[h] = stage_pool.tile(
                                [64, gsz * 512], F32, tag=f"stg{h}",
                                name=f"stg{h}",
                            )
                        ps = psum_pool.tile([64, 512], F32, tag="ps", name="ps")
                        # wm matmuls first (DVE produces wm before wp)
                        for ki, k in enumerate((0, 1, 3)):
                            dh, dw = DELTAS[k]
                            sm = (1 - dh) * 3 + (1 - dw)
                            nc.tensor.matmul(
                                ps[:, :],
                                wT[hr0 : hr0 + 64, sm * O : (sm + 1) * O],
                                wminus[k][hr0 : hr0 + 64, q : q + 512],
                                start=(ki == 0), stop=False,
                            )
                        for k, (dh, dw) in enumerate(DELTAS):
                            sp = (1 + dh) * 3 + (1 + dw)
                            nc.tensor.matmul(
                                ps[:, :],
                                wT[hr0 : hr0 + 64, sp * O : (sp + 1) * O],
                                wplus[k][hr0 : hr0 + 64, q : q + 512],
                                start=False, stop=False,
                            )
                        # merged (wm_2 | center) K=128 matmul last: its x
                        # copy arrives latest in the supply chain
                        tsl, ttile = (9, TA) if h == 0 else (10, TB)
                        nc.tensor.matmul(
                            ps[:, :], wT[0:128, tsl * O : (tsl + 1) * O],
                            ttile[0:128, q : q + 512],
                            start=False, stop=True,
                        )
                        nc.scalar.activation(
                            stages[h][:, (j % gsz) * 512 : (j % gsz + 1) * 512],
                            ps[:, :],
                            mybir.ActivationFunctionType.Identity,
                            bias=bias_col[:, :],
                        )
                        if j % gsz == gsz - 1:
                            nr = 4 * gsz
                            r0 = 64 * h + 4 * (j - gsz + 1)
                            # final stores on the HW DGE queues: the Pool
                            # sequencer's ~1us SWDGE issue would sit on the
                            # critical tail
                            dma = (
                                nc.gpsimd if j < 14
                                else (nc.sync if h == 0 else nc.scalar)
                            )
                            dma.dma_start(
                                out=out_t[:, r0 : r0 + nr, :],
                                in_=stages[h][:, :].rearrange(
                                    "o (r w) -> o r w", r=nr
                                ),
                            )

    nc.compile()
    return nc


_NC_CACHE = None
_WARMED = False


def _get_nc():
    global _NC_CACHE
    if _NC_CACHE is None:
        _NC_CACHE = build_program()
    return _NC_CACHE


def kernel(x, depth, weight, bias):
    x = np.asarray(x, dtype=np.float32)
    depth = np.asarray(depth, dtype=np.float32)
    weight = np.asarray(weight, dtype=np.float32)
    bias = np.asarray(bias, dtype=np.float32)
    B = x.shape[0]
    assert B == 8
    nc = _get_nc()
    in_maps = [
        {"x": x[b], "depth": depth[b], "weight": weight, "bias": bias}
        for b in range(B)
    ]
    # First execution after NEFF load can race the ACT table load on HW;
    # run once to warm up, then run for real.
    global _WARMED
    if not _WARMED:
        run_bass_kernel_spmd(nc, in_maps, core_ids=list(range(B)))
        _WARMED = True
    res = run_bass_kernel_spmd(nc, in_maps, core_ids=list(range(B)))
    return np.stack([res.results[b]["out"] for b in range(B)], axis=0)


if __name__ == "__main__":
    rng = np.random.default_rng(0)
    x = rng.standard_normal((8, C, H, W), dtype=np.float32)
    d = rng.random((8, 1, H, W), dtype=np.float32)
    w = rng.standard_normal((O, C, KH, KW), dtype=np.float32) * 0.04
    b = rng.standard_normal((O,), dtype=np.float32) * 0.04
    out = kernel(x=x, depth=d, weight=w, bias=b)
    print(out.shape, out.dtype)


# revision 73
# speedup vs baseline: 1.0018x; 1.0018x over previous
"""Depth-guided 3x3 convolution (nn_DepthConv) on 8 TRN2 NeuronCores.

Sharding: data-parallel over batch (B=8 -> 1 image per core). Weights/bias
replicated. No collectives.

Per-core algorithm (image [C=64, H=128, W=128]):
  out[o,p] = bias[o] + sum_t W_t[o,c] * x[c, p+dt] * exp(-|d[p+dt]-d[p]|)

Layout (v2): columns packed at W=128 (NO column padding). Column wrap-around
of the dw=+-1 taps is killed by forcing the corresponding sim-map column to
zero (reference pads x with zeros, so a zero contribution is exact).
Rows split in halves on partitions: [0:64] = channels for image rows -1..64
(frame A), [64:128] = rows 63..128 (frame B); 66 frame rows -> FR = 66*128.
Frame row f=0 of A is the zero pad row; f=65 of B likewise.

With m_d[q] := exp(-|D(q+d)-D(q)|) (computed pixel-major [row, col] on 128
row-partitions, flattened into HBM scratch rows, then broadcast into the wp
tiles -- one DMA per map covers both halves via a 2-level partition AP):
  wp_d[q] = x[q+dl]*m_d[q]       (in-place over the broadcast map)
  wm_d[q] = x[q-dl]*m_d[q-dl]    (reads wp's pristine map slot shifted)
  center tap reads x directly (sim == 1).

Matmuls: per output chunk (4 rows, N=512) 9 fp16 matmuls (K=64) accumulate
into a [64, 512] PSUM tile; Act evacuates with fused bias into 4-chunk stage
tiles; SWDGE stores them. All bulk transfers use >=512B descriptors. The
pipeline runs in chunk-bands (small at both ends) so map-broadcast / x-load
/ multiply / matmul / store overlap and the PE stays saturated.
"""

import sys

sys.path.insert(0, "/opt/trn_rl_repo")

import numpy as np

import bass_rust
import concourse.bass as bass
import concourse.mybir as mybir
import concourse.bacc as bacc
import concourse.tile as tile
from concourse.bass_utils import run_bass_kernel_spmd
from concourse.masks import make_identity

F16 = mybir.dt.float16
F32 = mybir.dt.float32

C, O, H, W, KH, KW = 64, 64, 128, 128, 3, 3
ALPHA = 1.0
RH = 66               # frame rows per half (A: input rows -1..64, B: 63..128)
FR = RH * W           # 8448
DELTAS = [(0, 1), (1, -1), (1, 0), (1, 1)]
DLS = [dh * W + dw for dh, dw in DELTAS]   # 1, 127, 128, 129
ZCOL = {0: 127, 1: 0, 3: 127}              # forced zero column per map
NCH = 16              # output chunks per half (4 rows each)
Q0 = 128              # first output flat index (frame row 1)
QE = Q0 + NCH * 512   # 8320

# chunk-range bands (per half): small at both ends for pipeline ramp
BANDS = [(0, 3), (3, 5), (5, 7), (7, 9), (9, 12), (12, 15), (15, 16)]


def build_program():
    nc = bacc.Bacc("TRN2", target_bir_lowering=False, debug=False)

    x_t = nc.dram_tensor("x", [C, H, W], F32, kind="ExternalInput")
    d_t = nc.dram_tensor("depth", [1, H, W], F32, kind="ExternalInput")
    w_t = nc.dram_tensor("weight", [O, C, KH, KW], F32, kind="ExternalInput")
    b_t = nc.dram_tensor("bias", [O], F32, kind="ExternalInput")
    out_t = nc.dram_tensor("out", [O, H, W], F32, kind="ExternalOutput")
    # map scratch: row 2k = map k half A, row 2k+1 = half B (adjacent rows so
    # one flatten DMA writes both and one broadcast DMA reads both)
    scratch = nc.dram_tensor("mscratch", [8, FR], F16, kind="Internal")

    def bc_ap(k, w0, w1):
        """[128, w1-w0] read: scratch row 2k 64x broadcast, then row 2k+1."""
        base = scratch[2 * k, w0:w1]
        return bass_rust.AP(
            tensor=base.tensor, offset=base.offset,
            ap=[[FR, 2], [0, 64], [1, w1 - w0]],
        )

    with tile.TileContext(nc) as tc:
        with (
            tc.tile_pool(name="big", bufs=1) as big,
            tc.tile_pool(name="small", bufs=1) as small,
            tc.tile_pool(name="mapp", bufs=2) as mapp,
            tc.tile_pool(name="psum", bufs=6, space="PSUM") as psum_pool,
            tc.tile_pool(name="psumw", bufs=1, space="PSUM") as psumw_pool,
            tc.tile_pool(name="stage", bufs=2) as stage_pool,
        ):
            # ---------------- persistent SBUF tensors ----------------
            xbuf = big.tile([128, FR], F16, tag="xbuf")
            wplus = [
                big.tile([128, FR], F16, tag=f"wp{k}", name=f"wp{k}")
                for k in range(4)
            ]
            # wm_2 is merged with the center tap into K=128 matmul tiles:
            # TA = (wm_2^A | x_A), TB = (x_B | wm_2^B) -- the x copy sits on
            # the partition range where DVE/map-slot alignment works out.
            wminus = [
                big.tile([128, FR], F16, tag=f"wm{k}", name=f"wm{k}")
                if k != 2 else None
                for k in range(4)
            ]
            TA = big.tile([128, FR], F16, tag="TA")
            TB = big.tile([128, FR], F16, tag="TB")
            # slots 0-8: taps; slot 9: [W_-d2 ; W_cc] (TA), 10: [W_cc ; W_-d2]
            wT = small.tile([128, 11 * O], F16, tag="wT")
            # w_t loaded twice side by side: one [64,128] transpose per tap
            # (strided in-AP over both copies) fills both wT halves at once
            w_raw = small.tile([64, 2 * 576], F32, tag="wraw")
            bias_col = small.tile([64, 1], F32, tag="bias")
            dbuf = small.tile([128, W], F32, tag="dbuf")
            dsh = small.tile([128, W], F32, tag="dsh")
            ident = small.tile([128, 64], F32, tag="ident")
            zrow = small.tile([4, W], F16, tag="zrow")

            # ---------------- warm ACT tables (Abs/Exp) ----------------
            warm = small.tile([1, 8], F32, tag="warm")
            nc.vector.memset(warm[:, :], 0.0)
            nc.scalar.activation(
                warm[:, :], warm[:, :], mybir.ActivationFunctionType.Abs
            )
            nc.scalar.activation(
                warm[:, :], warm[:, :], mybir.ActivationFunctionType.Exp
            )

            # ---------------- maps path first (critical chain) ----------------
            # depth + weights on the sync queue (the warm-up blocks scalar
            # ~2.2us); everything here is small and must transfer early
            nc.sync.dma_start(out=dbuf[0:128, :], in_=d_t[0, :, :])
            nc.vector.memset(dsh[:, :], 0.0)
            nc.sync.dma_start(out=dsh[0:127, :], in_=d_t[0, 1:128, :])
            nc.sync.dma_start(out=w_raw[:, 0:576], in_=w_t[:, :, :, :])
            nc.sync.dma_start(out=w_raw[:, 576:1152], in_=w_t[:, :, :, :])
            nc.vector.memset(zrow[:, :], 0.0)
            nc.scalar.dma_start(
                out=bias_col[0:64, 0:1],
                in_=b_t[:].rearrange("(p o) -> p o", o=1),
            )

            # phase 1: all depth diffs back-to-back on DVE
            diffs = []
            for k, (dh, dw) in enumerate(DELTAS):
                dsrc = dsh if dh == 1 else dbuf
                a = max(0, -dw)
                b = min(W, W - dw)
                diff = mapp.tile([128, W], F32, tag=f"diff{k}", name=f"diff{k}")
                if dw != 0:
                    nc.vector.memset(diff[:, :], 0.0)
                nc.vector.tensor_sub(
                    diff[:, a:b], dsrc[:, a + dw : b + dw], dbuf[:, a:b]
                )
                diffs.append(diff)
            # x-load gate: zero the first element of EVERY frame row after
            # the subs on DVE. Every x load overlaps some row start (WAW), so
            # all of x queues at the DMA engines behind the latency-critical
            # map flatten/broadcast transfers. The cells are overwritten by
            # the loads (or by the halo zero rows).
            xrows = xbuf[:, :].rearrange("p (r w) -> p r w", r=RH)
            nc.vector.memset(xrows[:, :, 0:1], 0.0)
            tarows = TA[:, :].rearrange("p (r w) -> p r w", r=RH)
            nc.vector.memset(tarows[:, :, 0:1], 0.0)
            tbrows = TB[:, :].rearrange("p (r w) -> p r w", r=RH)
            nc.vector.memset(tbrows[:, :, 0:1], 0.0)

            # phase 2: abs+exp on Act, zero columns on DVE, flatten DMAs
            for k in range(4):
                absd = mapp.tile([128, W], F32, tag="absd")
                nc.scalar.activation(
                    absd[:, :], diffs[k][:, :], mybir.ActivationFunctionType.Abs
                )
                mt = mapp.tile([128, W], F16, tag=f"mt{k}")
                nc.scalar.activation(
                    mt[:, :], absd[:, :],
                    mybir.ActivationFunctionType.Exp, scale=-ALPHA,
                )
                if k in ZCOL:
                    nc.vector.memset(mt[:, ZCOL[k] : ZCOL[k] + 1], 0.0)

                # flatten to scratch rows 2k (half A, map rows -1..64 with
                # row -1 zeroed) and 2k+1 (half B, rows 63..128)
                dstA = scratch[2 * k : 2 * k + 1, 128:FR].rearrange(
                    "p (r w) -> p r w", r=65
                )
                dma = nc.sync if k % 2 == 0 else nc.scalar
                dma.dma_start(out=dstA[:, :, :], in_=mt[0:65, :])
                dstB = scratch[2 * k + 1 : 2 * k + 2, 0 : 65 * W].rearrange(
                    "p (r w) -> p r w", r=65
                )
                dma = nc.scalar if k % 2 == 0 else nc.sync
                dma.dma_start(out=dstB[:, :, :], in_=mt[63:128, :])

            # zero the 4 half-A slot heads (map "row -1") in one DMA
            zdst = scratch[0, 0:W]
            zout = bass_rust.AP(
                tensor=zdst.tensor, offset=zdst.offset, ap=[[2 * FR, 4], [1, W]]
            )
            nc.scalar.dma_start(out=zout, in_=zrow[0:4, :])

            # ---------------- weights -> lhsT [c, (t, o)] fp16 ----------------
            # duplicated load; one [64,128] transpose per tap fills both wT
            # partition halves (no SBUF->SBUF DMA, which would queue behind
            # bulk transfers on the DMA engines)
            make_identity(nc, ident[0:64, :])
            # all 9 transposes into 2 PSUM tiles (no pool-recycle stalls),
            # then bulk psum->wT copies on DVE (Act is busy with maps)
            wps1 = psumw_pool.tile([128, 5 * O], F32, tag="wtp1", name="wps1")
            wps2 = psumw_pool.tile([128, 4 * O], F32, tag="wtp2", name="wps2")
            for t in range(9):
                wr = w_raw[0:1, t : t + 1]
                win = bass_rust.AP(
                    tensor=wr.tensor, offset=wr.offset,
                    ap=[[1152, 64], [576, 2], [9, 64]],
                )
                dst = (
                    wps1[:, t * O : (t + 1) * O]
                    if t < 5
                    else wps2[:, (t - 5) * O : (t - 4) * O]
                )
                nc.tensor.transpose(dst, win, ident[0:64, :])
            nc.vector.tensor_copy(out=wT[:, 0 : 5 * O], in_=wps1[:, :])
            nc.vector.tensor_copy(out=wT[:, 5 * O : 9 * O], in_=wps2[:, :])
            # mixed slots for the merged (wm_2 | center) matmuls;
            # tap t=1 is W[-d2] (kh=0,kw=1), t=4 is the center
            nc.vector.tensor_copy(
                out=wT[0:64, 9 * O : 10 * O], in_=wps1[0:64, 1 * O : 2 * O]
            )
            nc.vector.tensor_copy(
                out=wT[64:128, 9 * O : 10 * O], in_=wps1[64:128, 4 * O : 5 * O]
            )
            nc.vector.tensor_copy(
                out=wT[0:64, 10 * O : 11 * O], in_=wps1[0:64, 4 * O : 5 * O]
            )
            nc.vector.tensor_copy(
                out=wT[64:128, 10 * O : 11 * O], in_=wps1[64:128, 1 * O : 2 * O]
            )

            # ---------------- halo zero rows / columns ----------------
            nc.gpsimd.memset(xbuf[0:64, 0:W], 0.0)            # A frame row 0
            nc.gpsimd.memset(xbuf[64:128, FR - W : FR], 0.0)  # B frame row 65
            # wp3 last output column: never written by its multiply but read
            # by matmuls; true value is 0 (map zero column kills the wrap)
            nc.gpsimd.memset(wplus[3][:, QE - 1 : QE], 0.0)
            # wm3[128] = x[-1]*m[-1] = 0: below the band-0 multiply range
            nc.gpsimd.memset(wminus[3][:, Q0 : DLS[3]], 0.0)


            # ---------------- banded pipeline ----------------
            xrow = [0]    # frame rows of x loaded so far (same for A and B)
            stages = [None, None]

            def load_x(upto_row):
                r0, r1 = xrow[0], min(RH, upto_row)
                if r1 <= r0:
                    return
                xrow[0] = r1
                ra0, ra1 = max(r0, 1), r1      # A rows with data (row 0 = pad)
                rb0, rb1 = r0, min(r1, 65)     # B rows with data (row 65 = pad)
                if ra0 == rb0 and ra1 == rb1:
                    # middle band: both halves in one DMA (2-level partitions)
                    xb = x_t[0, ra0 - 1, 0:W]
                    src = bass_rust.AP(
                        tensor=xb.tensor, offset=xb.offset,
                        ap=[[64 * W, 2], [H * W, 64], [W, ra1 - ra0], [1, W]],
                    )
                    nc.gpsimd.dma_start(
                        out=xbuf[0:128, ra0 * W : ra1 * W], in_=src
                    )
                else:
                    nc.gpsimd.dma_start(
                        out=xbuf[0:64, ra0 * W : ra1 * W],
                        in_=x_t[:, ra0 - 1 : ra1 - 1, :],
                    )
                    nc.gpsimd.dma_start(
                        out=xbuf[64:128, rb0 * W : rb1 * W],
                        in_=x_t[:, rb0 + 63 : rb1 + 63, :],
                    )
                # x copies for the merged-tap tiles (center-tap halves):
                # SBUF->SBUF from xbuf (no cast -> HWDGE queues, keeping the
                # Pool sequencer free for the xbuf loads)
                nc.sync.dma_start(
                    out=TA[64:128, ra0 * W : ra1 * W],
                    in_=xbuf[0:64, ra0 * W : ra1 * W],
                )
                nc.scalar.dma_start(
                    out=TB[0:64, rb0 * W : rb1 * W],
                    in_=xbuf[64:128, rb0 * W : rb1 * W],
                )

            bq = [0]  # broadcast cursor
            for bi, (j0, j1) in enumerate(BANDS):
                s = Q0 + 512 * j0
                e = Q0 + 512 * j1
                last = j1 == NCH

                # map broadcasts for window [bq, e): one DMA per map
                w0, w1 = bq[0], e
                bq[0] = e
                for k in range(4):
                    w1k = min(w1, QE - 1) if k == 3 else w1
                    dma = nc.sync if k % 2 == 0 else nc.scalar
                    dma.dma_start(
                        out=wplus[k][0:128, w0:w1k], in_=bc_ap(k, w0, w1k)
                    )

                # x rows needed: wp reads up to e + 129 -> row e//W + 2
                load_x(RH if last else e // W + 2)

                # wm_k[q] = x[q-dl]*m_k[q-dl], q in [s+dl, e+dl) (band-shifted
                # so reads hit the pristine map slot of this band's window)
                # band 0: per-chunk windows so the first matmuls start early
                subw = (
                    [(Q0 + 512 * j, Q0 + 512 * (j + 1)) for j in range(j0, j1)]
                    if bi == 0 else [(s, e)]
                )
                for swi, (s, e) in enumerate(subw):
                  for k in range(4):
                    dl = DLS[k]
                    a = max(Q0, dl) if (bi == 0 and swi == 0) else s + dl
                    b = min(e + dl, QE)
                    if k == 2:
                        # wm_2 halves live in TA[0:64] / TB[64:128]
                        nc.vector.tensor_tensor(
                            out=TA[0:64, a:b],
                            in0=xbuf[0:64, a - dl : b - dl],
                            in1=wplus[2][0:64, a - dl : b - dl],
                            op=mybir.AluOpType.mult,
                        )
                        nc.vector.tensor_tensor(
                            out=TB[64:128, a:b],
                            in0=xbuf[64:128, a - dl : b - dl],
                            in1=wplus[2][64:128, a - dl : b - dl],
                            op=mybir.AluOpType.mult,
                        )
                        continue
                    nc.vector.tensor_tensor(
                        out=wminus[k][:, a:b],
                        in0=xbuf[:, a - dl : b - dl],
                        in1=wplus[k][:, a - dl : b - dl],
                        op=mybir.AluOpType.mult,
                    )
                  # wp_k[q] = x[q+dl]*m_k[q] in place, q in [s, e)
                  for k in range(4):
                    dl = DLS[k]
                    b = min(e, FR - dl)
                    nc.vector.tensor_mul(
                        wplus[k][:, s:b], xbuf[:, s + dl : b + dl],
                        wplus[k][:, s:b],
                    )

                # matmuls + evac (+ store per 4-chunk group)
                for j in range(j0, j1):
                    q = Q0 + 512 * j
                    for h in range(2):
                        hr0 = 64 * h
                        # smaller groups toward the end -> short tail
                        gsz = 2 if j < 14 else 1
                        if j % gsz == 0:
                            stages[h] = stage_pool.tile(
                                [64, gsz * 512], F32, tag=f"stg{h}",
                                name=f"stg{h}",
                            )
                        ps = psum_pool.tile([64, 512], F32, tag="ps", name="ps")
                        # wm matmuls first (DVE produces wm before wp)
                        for ki, k in enumerate((0, 1, 3)):
                            dh, dw = DELTAS[k]
                            sm = (1 - dh) * 3 + (1 - dw)
                            nc.tensor.matmul(
                                ps[:, :],
                                wT[hr0 : hr0 + 64, sm * O : (sm + 1) * O],
                                wminus[k][hr0 : hr0 + 64, q : q + 512],
                                start=(ki == 0), stop=False,
                            )
                        for k, (dh, dw) in enumerate(DELTAS):
                            sp = (1 + dh) * 3 + (1 + dw)
                            nc.tensor.matmul(
                                ps[:, :],
                                wT[hr0 : hr0 + 64, sp * O : (sp + 1) * O],
                                wplus[k][hr0 : hr0 + 64, q : q + 512],
                                start=False, stop=False,
                            )
                        # merged (wm_2 | center) K=128 matmul last: its x
                        # copy arrives latest in the supply chain
                        tsl, ttile = (9, TA) if h == 0 else (10, TB)
                        nc.tensor.matmul(
                            ps[:, :], wT[0:128, tsl * O : (tsl + 1) * O],
                            ttile[0:128, q : q + 512],
                            start=False, stop=True,
                        )
                        nc.scalar.activation(
                            stages[h][:, (j % gsz) * 512 : (j % gsz + 1) * 512],
                            ps[:, :],
                            mybir.ActivationFunctionType.Identity,
                            bias=bias_col[:, :],
                        )
                        if j % gsz == gsz - 1:
                            nr = 4 * gsz
                            r0 = 64 * h + 4 * (j - gsz + 1)
                            # final stores on the HW DGE queues: the Pool
                            # sequencer's ~1us SWDGE issue would sit on the
                            # critical tail
                            dma = (
                                nc.gpsimd if j < 14
                                else (nc.sync if h == 0 else nc.scalar)
                            )
                            dma.dma_start(
                                out=out_t[:, r0 : r0 + nr, :],
                                in_=stages[h][:, :].rearrange(
                                    "o (r w) -> o r w", r=nr
                                ),
                            )

    nc.compile()
    return nc


_NC_CACHE = None
_WARMED = False


def _get_nc():
    global _NC_CACHE
    if _NC_CACHE is None:
        _NC_CACHE = build_program()
    return _NC_CACHE


def kernel(x, depth, weight, bias):
    x = np.asarray(x, dtype=np.float32)
    depth = np.asarray(depth, dtype=np.float32)
    weight = np.asarray(weight, dtype=np.float32)
    bias = np.asarray(bias, dtype=np.float32)
    B = x.shape[0]
    assert B == 8
    nc = _get_nc()
    in_maps = [
        {"x": x[b], "depth": depth[b], "weight": weight, "bias": bias}
        for b in range(B)
    ]
    # First execution after NEFF load can race the ACT table load on HW;
    # run once to warm up, then run for real.
    global _WARMED
    if not _WARMED:
        run_bass_kernel_spmd(nc, in_maps, core_ids=list(range(B)))
        _WARMED = True
    res = run_bass_kernel_spmd(nc, in_maps, core_ids=list(range(B)))
    return np.stack([res.results[b]["out"] for b in range(B)], axis=0)


if __name__ == "__main__":
    rng = np.random.default_rng(0)
    x = rng.standard_normal((8, C, H, W), dtype=np.float32)
    d = rng.random((8, 1, H, W), dtype=np.float32)
    w = rng.standard_normal((O, C, KH, KW), dtype=np.float32) * 0.04
    b = rng.standard_normal((O,), dtype=np.float32) * 0.04
    out = kernel(x=x, depth=d, weight=w, bias=b)
    print(out.shape, out.dtype)
